# revision 33
# baseline (speedup 1.0000x reference)
"""LoRA first-layer MLP kernel for 8 Trainium2 NeuronCores.

Computation:
    W_eff = W0 + 2.0 * (B @ A)            # [4096, 1024]  (folded on host)
    h     = relu(x @ W_eff^T + b0)        # [16384, 4096]
    out   = (h @ W2^T + b2).squeeze(-1)   # [16384]

Sharding: data-parallel over batch; each of the 8 cores handles 2048 rows of
x and replicates the weights. No collectives needed.

Per-core device kernel:
  - W_eff is merged + cast to bf16 on the host (rel err ~1.5e-3, well under
    the 2e-2 gate); x is cast to bf16 too. Halves HBM traffic vs fp32r and
    keeps the PE at 1 cycle/row (cost model keys on the moving dtype).
  - Layer 1: h^T[m, b] tiles [128, 512] accumulated on PE over 8 d-chunks
    (lhsT = W_eff^T slice [128d, 128m], rhs = x^T slice [128d, 512b]).
  - relu+bias on ScalarE; layer 2 (sum_m W2[m]*h[m,b]) on VectorE via
    scalar_tensor_tensor into one f32r accumulator per batch chunk;
    partition-reduce via a ones-vector matmul deferred into the next chunk.
    The very last m-tile of the last chunk instead reduces on the PE
    (W2-column lhsT x bf16 relu output) to shorten the critical tail.
  - A run of dummy PE matmuls on a memset scratch tile covers the initial
    DMA wait so the PE p-state ramp (0.65/1.2 GHz until ~3us of continuous
    busy) is burned before the first real matmul.
  - DMA pacing: the 16-engine DMA pool fair-shares across all in-flight
    logical DMAs, so non-critical transfers are chained behind critical
    ones via 1-column WAW overlaps (each transfer rewrites its
    predecessor's last column), keeping the startup-critical x0a/W0-tile0
    transfers at full bandwidth and pacing the rest just-in-time.
"""

import sys

sys.path.insert(0, "/opt/trn_rl_repo")

import ml_dtypes
import numpy as np

import concourse.bacc as bacc
import concourse.mybir as mybir
import concourse.tile as tile
from concourse.bass_utils import run_bass_kernel_spmd

F32 = mybir.dt.float32
F32R = mybir.dt.float32r
BF16 = mybir.dt.bfloat16
NPBF16 = ml_dtypes.bfloat16

N_CORES = 8
B_FULL, D, M, R = 16384, 1024, 4096, 16
SCALING = 2.0
BS = B_FULL // N_CORES  # 2048 rows per core
NB = BS // 512  # 4 batch chunks per core
ND = D // 128  # 8 d-chunks
NM = M // 128  # 32 m-chunks
NM2 = M // 512  # 8 m-blocks of 512

NDUMMY = 52  # PE warmup matmuls to burn the p-state ramp during DMA wait

_CACHE = {}


def _build_nc():
    nc = bacc.Bacc(
        "TRN2",
        target_bir_lowering=False,
        debug=False,
        num_devices=N_CORES,
    )
    # x slab: xt2[p, bc*4096 + dc*512 + b] = x[bc*512 + b, dc*128 + p]
    xt2 = nc.dram_tensor("xt2", [128, NB * 4096], BF16, kind="ExternalInput").ap()
    # W slab: wt2[p, g*4096 + t*1024 + dc*128 + j]
    #           = W_eff[g*512 + t*128 + j, dc*128 + p]
    wt2 = nc.dram_tensor("wt2", [128, NM2 * 4096], BF16, kind="ExternalInput").ap()
    bwp = nc.dram_tensor("bw", [128, 2 * NM], F32, kind="ExternalInput").ap()
    b2s = nc.dram_tensor("b2s", [1, 1], F32, kind="ExternalInput").ap()
    onesd = nc.dram_tensor("ones", [128, 1], F32R, kind="ExternalInput").ap()
    w2ld = nc.dram_tensor("w2l", [128, 1], BF16, kind="ExternalInput").ap()
    out = nc.dram_tensor("out", [1, BS], F32, kind="ExternalOutput").ap()

    RELU = mybir.ActivationFunctionType.Relu
    MULT = mybir.AluOpType.mult
    ADD = mybir.AluOpType.add

    with tile.TileContext(nc) as tc:
        with (
            tc.tile_pool(name="wp", bufs=1) as wp,
            tc.tile_pool(name="xp", bufs=1) as xp,
            tc.tile_pool(name="hb", bufs=4) as hb,
            tc.tile_pool(name="ab", bufs=2) as ab,
            tc.tile_pool(name="cp", bufs=1) as cp,
            tc.tile_pool(name="psh", bufs=3, space="PSUM") as psh,
            tc.tile_pool(name="pso", bufs=2, space="PSUM") as pso,
            tc.tile_pool(name="psd", bufs=1, space="PSUM") as psd,
        ):
            # PE warmup scratch: memset (no DMA dependency), then dummies.
            SCR = cp.tile([128, 128], BF16, tag="scr")
            nc.gpsimd.memset(SCR[:], 0.0)

            X = xp.tile([128, NB * 4096], BF16, tag="x")
            W = wp.tile([128, NM2 * 4096], BF16, tag="w")

            def dma(dst, dsrc, lo, hi, ov):
                """Copy cols [lo:hi) plus `ov` overlap cols (WAW chaining)."""
                nc.sync.dma_start(
                    out=dst[:, lo : hi + ov], in_=dsrc[:, lo : hi + ov]
                )

            # Unchained head wave (3MB, shares the DMA pool): x0 halves +
            # W0's four m-tiles. Gaps between their spans and the overlap
            # columns below chain everything later just-in-time so it does
            # not steal bus share from this critical set.
            dma(X, xt2, 0, 2048, 0)  # x0a (k-slices 0-3; gates the first MM)
            dma(W, wt2, 0, 1024, 0)  # W0 t0 (gates the first MM)
            dma(X, xt2, 2048, 4096, 1)  # x0b (+1 col: x1 chains on it)
            dma(W, wt2, 1024, 2048, 0)  # W0 t1
            dma(W, wt2, 2048, 3072, 0)  # W0 t2
            dma(W, wt2, 3072, 4096, 1)  # W0 t3 (+1 col: W1a chains on it)
            # W1-W3 chained in half-blocks: each ~2.3us link dead time +
            # 1.6us transfer stays ahead of its 2-tile (3.5us) consumption.
            dma(W, wt2, 4096, 6144, 1)  # W1a (chained on W0 t3)
            dma(X, xt2, 4096, 8192, 1)  # x1 (chained on x0b)
            dma(W, wt2, 6144, 8192, 1)  # W1b (chained on W1a)
            dma(X, xt2, 8192, 12288, 1)  # x2 (chained on x1)
            dma(W, wt2, 8192, 10240, 1)  # W2a (chained on W1b)
            dma(W, wt2, 10240, 12288, 1)  # W2b
            dma(X, xt2, 12288, 16384, 0)  # x3 (chained on x2)
            dma(W, wt2, 12288, 14336, 1)  # W3a
            dma(W, wt2, 14336, 16384, 1)  # W3b
            for g in range(4, NM2):
                dma(W, wt2, g * 4096, (g + 1) * 4096, 1 if g < NM2 - 1 else 0)

            # Small constants via the scalar engine's queue.
            BW = cp.tile([128, 2 * NM], F32, tag="bw")
            nc.scalar.dma_start(out=BW[:], in_=bwp)
            ONES = cp.tile([128, 1], F32R, tag="ones")
            nc.scalar.dma_start(out=ONES[:], in_=onesd)
            # W2 column for the last m-tile's PE-side reduce (bf16 to match
            # the bf16 relu output; walrus rejects f32r/bf16 mixing).
            W2L = cp.tile([128, 1], BF16, tag="w2l")
            nc.scalar.dma_start(out=W2L[:], in_=w2ld)
            B2 = cp.tile([1, 1], F32, tag="b2")
            nc.scalar.dma_start(out=B2[:], in_=b2s)

            # PE p-state warmup on scratch (independent of all DMAs).
            DP = psd.tile([128, 128], F32, tag="dp")
            for _ in range(NDUMMY):
                nc.tensor.matmul(DP[:], SCR[:], SCR[:], start=True, stop=True)

            pending_reduce = []

            def emit_reduce(bc, acc):
                op = pso.tile([1, 512], F32, tag="op")
                nc.tensor.matmul(op[:], ONES[:], acc[:], start=True, stop=True)
                os_t = ab.tile([1, 512], F32, tag="os")
                nc.vector.tensor_scalar_add(os_t[:], op[:], B2[:, 0:1])
                nc.sync.dma_start(
                    out=out[:, bc * 512 : (bc + 1) * 512], in_=os_t[:]
                )

            for bc in range(NB):
                acc = ab.tile([128, 512], F32R, tag="acc")
                last = bc == NB - 1
                for mc in range(NM):
                    g, t = mc // 4, mc % 4
                    if mc == 2 and pending_reduce:
                        emit_reduce(*pending_reduce.pop())
                    hp = psh.tile([128, 512], F32, tag="hp")
                    base = g * 4096 + t * 1024
                    xoff = bc * 4096
                    for dc in range(ND):
                        nc.tensor.matmul(
                            hp[:],
                            W[:, base + dc * 128 : base + (dc + 1) * 128],
                            X[:, xoff + dc * 512 : xoff + (dc + 1) * 512],
                            start=(dc == 0),
                            stop=(dc == ND - 1),
                        )
                    if last and mc == NM - 1:
                        # Tail shortcut: reduce tiles 0-30 now (overlaps the
                        # relu below), then fold tile 31 in on the PE from a
                        # bf16 relu output — skips the DVE accumulate.
                        op = pso.tile([1, 512], F32, tag="op")
                        nc.tensor.matmul(
                            op[:], ONES[:, 0:1], acc[:], start=True, stop=False
                        )
                        hB = hb.tile([128, 512], BF16, tag="hB")
                        nc.scalar.activation(
                            hB[:], hp[:], RELU, bias=BW[:, mc : mc + 1]
                        )
                        nc.tensor.matmul(
                            op[:], W2L[:], hB[:], start=False, stop=True
                        )
                        os_t = ab.tile([1, 512], F32, tag="os")
                        nc.vector.tensor_scalar_add(os_t[:], op[:], B2[:, 0:1])
                        nc.sync.dma_start(
                            out=out[:, bc * 512 : (bc + 1) * 512], in_=os_t[:]
                        )
                        continue
                    h = hb.tile([128, 512], F32, tag="h")
                    nc.scalar.activation(h[:], hp[:], RELU, bias=BW[:, mc : mc + 1])
                    if mc == 0:
                        nc.vector.tensor_scalar_mul(
                            acc[:], h[:], BW[:, NM + mc : NM + mc + 1]
                        )
                    else:
                        nc.vector.scalar_tensor_tensor(
                            acc[:], h[:], BW[:, NM + mc : NM + mc + 1], acc[:],
                            MULT, ADD,
                        )
                if not last:
                    pending_reduce.append((bc, acc))
            while pending_reduce:
                emit_reduce(*pending_reduce.pop(0))

    nc.compile()
    return nc


def _prep_in_maps(x, W0, b0, A, B, W2, b2):
    W_eff = (W0 + SCALING * (B @ A)).astype(NPBF16)
    # wt2[p, ((g, t, dc, j))] = W_eff[g*512 + t*128 + j, dc*128 + p]
    wt2 = np.ascontiguousarray(
        W_eff.reshape(NM2, 4, 128, ND, 128)
        .transpose(4, 0, 1, 3, 2)  # p, g, t, dc, j
        .reshape(128, NM2 * 4096)
    )
    bw = np.empty((128, 2 * NM), dtype=np.float32)
    bw[:, :NM] = b0.reshape(NM, 128).T
    bw[:, NM:] = W2[0].reshape(NM, 128).T
    b2s = b2.reshape(1, 1).astype(np.float32)
    ones = np.ones((128, 1), dtype=np.float32)
    w2l = W2[0, (NM - 1) * 128 :].reshape(128, 1).astype(NPBF16)

    xbf = x.astype(NPBF16)
    in_maps = []
    for c in range(N_CORES):
        xs = xbf[c * BS : (c + 1) * BS]  # [2048, 1024]
        # xt2[p, bc*4096 + dc*512 + b] = xs[bc*512 + b, dc*128 + p]
        xt2 = np.ascontiguousarray(
            xs.reshape(NB, 512, ND, 128)
            .transpose(3, 0, 2, 1)  # p, bc, dc, b
            .reshape(128, NB * 4096)
        )
        in_maps.append(
            {
                "xt2": xt2,
                "wt2": wt2,
                "bw": bw,
                "b2s": b2s,
                "ones": ones,
                "w2l": w2l,
            }
        )
    return in_maps


def kernel(x, W0, b0, A, B, W2, b2, _trace=False, _trace_kwargs=None):
    x = np.asarray(x, dtype=np.float32)
    W0 = np.asarray(W0, dtype=np.float32)
    b0 = np.asarray(b0, dtype=np.float32)
    A = np.asarray(A, dtype=np.float32)
    B = np.asarray(B, dtype=np.float32)
    W2 = np.asarray(W2, dtype=np.float32)
    b2 = np.asarray(b2, dtype=np.float32)

    if "nc" not in _CACHE:
        _CACHE["nc"] = _build_nc()
    nc = _CACHE["nc"]

    in_maps = _prep_in_maps(x, W0, b0, A, B, W2, b2)
    res = run_bass_kernel_spmd(
        nc,
        in_maps,
        list(range(N_CORES)),
        trace=_trace,
        **(_trace_kwargs or {}),
    )
    out = np.concatenate([r["out"].reshape(BS) for r in res.results])
    if _trace:
        _CACHE["last_results"] = res
    return out.astype(np.float32)


# revision 34
# speedup vs baseline: 1.0624x; 1.0624x over previous
"""LoRA first-layer MLP kernel for 8 Trainium2 NeuronCores.

Computation:
    W_eff = W0 + 2.0 * (B @ A)            # [4096, 1024]  (folded on host)
    h     = relu(x @ W_eff^T + b0)        # [16384, 4096]
    out   = (h @ W2^T + b2).squeeze(-1)   # [16384]

Sharding: data-parallel over batch; each of the 8 cores handles 2048 rows of
x and replicates the weights. No collectives needed.

Per-core device kernel:
  - W_eff is merged + cast to bf16 on the host (rel err ~1.5e-3, well under
    the 2e-2 gate); x is cast to bf16 too. Halves HBM traffic vs fp32r and
    keeps the PE at 1 cycle/row (cost model keys on the moving dtype).
  - Layer 1: h^T[m, b] tiles [128, 512] accumulated on PE over 8 d-chunks
    (lhsT = W_eff^T slice [128d, 128m], rhs = x^T slice [128d, 512b]).
  - relu+bias on ScalarE; layer 2 (sum_m W2[m]*h[m,b]) on VectorE via
    scalar_tensor_tensor into one f32r accumulator per batch chunk;
    partition-reduce via a ones-vector matmul deferred into the next chunk.
    The very last m-tile of the last chunk instead reduces on the PE
    (W2-column lhsT x bf16 relu output) to shorten the critical tail.
  - A run of dummy PE matmuls on a memset scratch tile covers the initial
    DMA wait so the PE p-state ramp (0.65/1.2 GHz until ~3us of continuous
    busy) is burned before the first real matmul.
  - DMA pacing: the 16-engine DMA pool fair-shares across all in-flight
    logical DMAs, so non-critical transfers are chained behind critical
    ones via 1-column WAW overlaps (each transfer rewrites its
    predecessor's last column), keeping the startup-critical x0a/W0-tile0
    transfers at full bandwidth and pacing the rest just-in-time.
"""

import sys

sys.path.insert(0, "/opt/trn_rl_repo")

import ml_dtypes
import numpy as np

import concourse.bacc as bacc
import concourse.mybir as mybir
import concourse.tile as tile
from concourse.bass_utils import run_bass_kernel_spmd

F32 = mybir.dt.float32
F32R = mybir.dt.float32r
BF16 = mybir.dt.bfloat16
NPBF16 = ml_dtypes.bfloat16

N_CORES = 8
B_FULL, D, M, R = 16384, 1024, 4096, 16
SCALING = 2.0
BS = B_FULL // N_CORES  # 2048 rows per core
NB = BS // 512  # 4 batch chunks per core
ND = D // 128  # 8 d-chunks
NM = M // 128  # 32 m-chunks
NM2 = M // 512  # 8 m-blocks of 512

NDUMMY = 62  # PE warmup matmuls to burn the p-state ramp during DMA wait

_CACHE = {}


def _build_nc():
    nc = bacc.Bacc(
        "TRN2",
        target_bir_lowering=False,
        debug=False,
        num_devices=N_CORES,
    )
    # x slab: xt2[p, bc*4096 + dc*512 + b] = x[bc*512 + b, dc*128 + p]
    xt2 = nc.dram_tensor("xt2", [128, NB * 4096], BF16, kind="ExternalInput").ap()
    # W slab: wt2[p, g*4096 + t*1024 + dc*128 + j]
    #           = W_eff[g*512 + t*128 + j, dc*128 + p]
    wt2 = nc.dram_tensor("wt2", [128, NM2 * 4096], BF16, kind="ExternalInput").ap()
    bwp = nc.dram_tensor("bw", [128, 2 * NM], F32, kind="ExternalInput").ap()
    b2s = nc.dram_tensor("b2s", [1, 1], F32, kind="ExternalInput").ap()
    onesd = nc.dram_tensor("ones", [128, 1], F32R, kind="ExternalInput").ap()
    w2ld = nc.dram_tensor("w2l", [128, 1], BF16, kind="ExternalInput").ap()
    out = nc.dram_tensor("out", [1, BS], F32, kind="ExternalOutput").ap()

    RELU = mybir.ActivationFunctionType.Relu
    MULT = mybir.AluOpType.mult
    ADD = mybir.AluOpType.add

    with tile.TileContext(nc) as tc:
        with (
            tc.tile_pool(name="wp", bufs=1) as wp,
            tc.tile_pool(name="xp", bufs=1) as xp,
            tc.tile_pool(name="hb", bufs=4) as hb,
            tc.tile_pool(name="ab", bufs=2) as ab,
            tc.tile_pool(name="cp", bufs=1) as cp,
            tc.tile_pool(name="psh", bufs=3, space="PSUM") as psh,
            tc.tile_pool(name="pso", bufs=2, space="PSUM") as pso,
            tc.tile_pool(name="psd", bufs=1, space="PSUM") as psd,
        ):
            # PE warmup scratch: memset (no DMA dependency), then dummies.
            SCR = cp.tile([128, 128], BF16, tag="scr")
            nc.gpsimd.memset(SCR[:], 0.0)

            X = xp.tile([128, NB * 4096], BF16, tag="x")
            W = wp.tile([128, NM2 * 4096], BF16, tag="w")

            def dma(dst, dsrc, lo, hi, ov):
                """Copy cols [lo:hi) plus `ov` overlap cols (WAW chaining)."""
                nc.sync.dma_start(
                    out=dst[:, lo : hi + ov], in_=dsrc[:, lo : hi + ov]
                )

            # Unchained head wave (3MB, shares the DMA pool): x0 halves +
            # W0's four m-tiles. Gaps between their spans and the overlap
            # columns below chain everything later just-in-time so it does
            # not steal bus share from this critical set.
            dma(X, xt2, 0, 2048, 0)  # x0a (k-slices 0-3; gates the first MM)
            dma(W, wt2, 0, 1024, 0)  # W0 t0 (gates the first MM)
            dma(X, xt2, 2048, 4096, 1)  # x0b (+1 col: x1 chains on it)
            dma(W, wt2, 1024, 2048, 0)  # W0 t1
            dma(W, wt2, 2048, 3072, 0)  # W0 t2
            dma(W, wt2, 3072, 4096, 0)  # W0 t3
            dma(W, wt2, 4096, 8192, 1)  # W1 (unchained: deadline is tight)
            dma(X, xt2, 4096, 8192, 1)  # x1 (chained on x0b)
            dma(W, wt2, 8192, 12288, 1)  # W2 (chained on W1)
            dma(X, xt2, 8192, 12288, 1)  # x2 (chained on x1)
            dma(W, wt2, 12288, 16384, 1)  # W3 (chained on W2)
            dma(X, xt2, 12288, 16384, 0)  # x3 (chained on x2)
            for g in range(4, NM2):
                dma(W, wt2, g * 4096, (g + 1) * 4096, 1 if g < NM2 - 1 else 0)

            # Small constants via the scalar engine's queue.
            BW = cp.tile([128, 2 * NM], F32, tag="bw")
            nc.scalar.dma_start(out=BW[:], in_=bwp)
            ONES = cp.tile([128, 1], F32R, tag="ones")
            nc.scalar.dma_start(out=ONES[:], in_=onesd)
            # W2 column for the last m-tile's PE-side reduce (bf16 to match
            # the bf16 relu output; walrus rejects f32r/bf16 mixing).
            W2L = cp.tile([128, 1], BF16, tag="w2l")
            nc.scalar.dma_start(out=W2L[:], in_=w2ld)
            B2 = cp.tile([1, 1], F32, tag="b2")
            nc.scalar.dma_start(out=B2[:], in_=b2s)

            # PE p-state warmup on scratch (independent of all DMAs).
            DP = psd.tile([128, 128], F32, tag="dp")
            for _ in range(NDUMMY):
                nc.tensor.matmul(DP[:], SCR[:], SCR[:], start=True, stop=True)

            pending_reduce = []

            def emit_reduce(bc, acc):
                op = pso.tile([1, 512], F32, tag="op")
                nc.tensor.matmul(op[:], ONES[:], acc[:], start=True, stop=True)
                os_t = ab.tile([1, 512], F32, tag="os")
                nc.vector.tensor_scalar_add(os_t[:], op[:], B2[:, 0:1])
                nc.sync.dma_start(
                    out=out[:, bc * 512 : (bc + 1) * 512], in_=os_t[:]
                )

            for bc in range(NB):
                acc = ab.tile([128, 512], F32R, tag="acc")
                last = bc == NB - 1
                for mc in range(NM):
                    g, t = mc // 4, mc % 4
                    if mc == 2 and pending_reduce:
                        emit_reduce(*pending_reduce.pop())
                    hp = psh.tile([128, 512], F32, tag="hp")
                    base = g * 4096 + t * 1024
                    xoff = bc * 4096
                    for dc in range(ND):
                        nc.tensor.matmul(
                            hp[:],
                            W[:, base + dc * 128 : base + (dc + 1) * 128],
                            X[:, xoff + dc * 512 : xoff + (dc + 1) * 512],
                            start=(dc == 0),
                            stop=(dc == ND - 1),
                        )
                    if last and mc == NM - 1:
                        # Tail shortcut: reduce tiles 0-30 now (overlaps the
                        # relu below), then fold tile 31 in on the PE from a
                        # bf16 relu output — skips the DVE accumulate.
                        op = pso.tile([1, 512], F32, tag="op")
                        nc.tensor.matmul(
                            op[:], ONES[:, 0:1], acc[:], start=True, stop=False
                        )
                        hB = hb.tile([128, 512], BF16, tag="hB")
                        nc.scalar.activation(
                            hB[:], hp[:], RELU, bias=BW[:, mc : mc + 1]
                        )
                        nc.tensor.matmul(
                            op[:], W2L[:], hB[:], start=False, stop=True
                        )
                        os_t = ab.tile([1, 512], F32, tag="os")
                        nc.vector.tensor_scalar_add(os_t[:], op[:], B2[:, 0:1])
                        nc.sync.dma_start(
                            out=out[:, bc * 512 : (bc + 1) * 512], in_=os_t[:]
                        )
                        continue
                    h = hb.tile([128, 512], F32, tag="h")
                    nc.scalar.activation(h[:], hp[:], RELU, bias=BW[:, mc : mc + 1])
                    if mc == 0:
                        nc.vector.tensor_scalar_mul(
                            acc[:], h[:], BW[:, NM + mc : NM + mc + 1]
                        )
                    else:
                        nc.vector.scalar_tensor_tensor(
                            acc[:], h[:], BW[:, NM + mc : NM + mc + 1], acc[:],
                            MULT, ADD,
                        )
                if not last:
                    pending_reduce.append((bc, acc))
            while pending_reduce:
                emit_reduce(*pending_reduce.pop(0))

    nc.compile()
    return nc


def _prep_in_maps(x, W0, b0, A, B, W2, b2):
    W_eff = (W0 + SCALING * (B @ A)).astype(NPBF16)
    # wt2[p, ((g, t, dc, j))] = W_eff[g*512 + t*128 + j, dc*128 + p]
    wt2 = np.ascontiguousarray(
        W_eff.reshape(NM2, 4, 128, ND, 128)
        .transpose(4, 0, 1, 3, 2)  # p, g, t, dc, j
        .reshape(128, NM2 * 4096)
    )
    bw = np.empty((128, 2 * NM), dtype=np.float32)
    bw[:, :NM] = b0.reshape(NM, 128).T
    bw[:, NM:] = W2[0].reshape(NM, 128).T
    b2s = b2.reshape(1, 1).astype(np.float32)
    ones = np.ones((128, 1), dtype=np.float32)
    w2l = W2[0, (NM - 1) * 128 :].reshape(128, 1).astype(NPBF16)

    xbf = x.astype(NPBF16)
    in_maps = []
    for c in range(N_CORES):
        xs = xbf[c * BS : (c + 1) * BS]  # [2048, 1024]
        # xt2[p, bc*4096 + dc*512 + b] = xs[bc*512 + b, dc*128 + p]
        xt2 = np.ascontiguousarray(
            xs.reshape(NB, 512, ND, 128)
            .transpose(3, 0, 2, 1)  # p, bc, dc, b
            .reshape(128, NB * 4096)
        )
        in_maps.append(
            {
                "xt2": xt2,
                "wt2": wt2,
                "bw": bw,
                "b2s": b2s,
                "ones": ones,
                "w2l": w2l,
            }
        )
    return in_maps


def kernel(x, W0, b0, A, B, W2, b2, _trace=False, _trace_kwargs=None):
    x = np.asarray(x, dtype=np.float32)
    W0 = np.asarray(W0, dtype=np.float32)
    b0 = np.asarray(b0, dtype=np.float32)
    A = np.asarray(A, dtype=np.float32)
    B = np.asarray(B, dtype=np.float32)
    W2 = np.asarray(W2, dtype=np.float32)
    b2 = np.asarray(b2, dtype=np.float32)

    if "nc" not in _CACHE:
        _CACHE["nc"] = _build_nc()
    nc = _CACHE["nc"]

    in_maps = _prep_in_maps(x, W0, b0, A, B, W2, b2)
    res = run_bass_kernel_spmd(
        nc,
        in_maps,
        list(range(N_CORES)),
        trace=_trace,
        **(_trace_kwargs or {}),
    )
    out = np.concatenate([r["out"].reshape(BS) for r in res.results])
    if _trace:
        _CACHE["last_results"] = res
    return out.astype(np.float32)
